# revision 33
# baseline (speedup 1.0000x reference)
"""Trainium2 Bass kernel for nn_EdgeEncoder (moe_routing).

Strategy
--------
Each of E edges is routed to 1 of 9 expert MLPs (4 -> 256 -> 256), then
  out = relu(concat([type_embed[tid], source_embed[sid], pv]) @ Wf + bf).

Host (numpy, cheap O(E) work):
  * scale/mask params; group edges by expert (base type), and within an
    expert by (type_id, source_id) pair; DEAL each pair's edges round-robin
    across the 8 cores so every core sees identical pair-run boundaries
    (ceil(n_pair/8) columns each) -> one compiled program serves all cores,
  * algebraic fusions so the device does minimal work:
      - x gets a ones-row so b1 rides inside the layer-1 matmul,
      - V[t] = W2[t] @ Wf_pv (f64 host precompute) fuses layer 2 with the
        final projection: pv @ Wf_pv == h @ V[t] + const,
      - the whole "embedding" term  C[tid,sid] = type_embed[tid] @ Wf_t
        + source_embed[sid] @ Wf_s + b2[t] @ Wf_pv + bf  is constant per
        (tid,sid) pair.  Because edges are sorted by pair, C is piecewise
        constant along the edge axis and rides FOR FREE as the per-partition
        bias operand of the PSUM->SBUF relu evacuation ops.  This deletes
        the K=20 one-hot matmuls entirely (8 -> 6 PE passes per block).

Device per 512-edge block (edges pre-grouped, transposed):
  hT  = relu(W1e[t].T @ xT1)           2 matmuls fp32r K=5(pad 128) N=512
  outT[g] += V[t]-chunk(h).T @ hT[h]   4 matmuls bf16  K=128        N=512
  relu(outT + C[pair]) PSUM -> bf16 SBUF (bias-relu ops split between the
  Scalar and Vector engines, one slice per pair-run), DMA outT tiles to
  DRAM [D, L] bf16; host un-permutes and upcasts.

All matmuls are full 128-row (inputs zero-padded): partial-K matmuls run
in PE tile mode and starve the HAM activity monitor, dropping the PE to
1.2 GHz.  A short bf16 warm-up burst raises the HAM clock gate at kernel
start while the first DMAs are in flight.
"""

import math
import os

import ml_dtypes
import numpy as np

import concourse.bacc as bacc
import concourse.bass as bass
import concourse.mybir as mybir
import concourse.tile as tile
from concourse.bass_utils import run_bass_kernel_spmd

# ---- static module configuration (mirrors the torch source) ----
T = 9            # base types ("experts")
P_MAX = 4
D = 256
N_TYPES = 14
N_SRC = 5
NCORES = 8
BLOCK = 512      # edges per device block (one PSUM bank of fp32)
GRP = 8          # expert segment granularity (PSUM 8-byte-line alignment)
PDIM = 128       # PE partition dim

BASE_MAP = np.array([0, 0, 0, 1, 1, 1, 2, 2, 3, 4, 5, 6, 7, 8], dtype=np.int32)
PCOUNT = np.array([2, 2, 1, 1, 1, 1, 3, 2, 4], dtype=np.int32)
SCALES = np.ones((T, P_MAX), dtype=np.float32)
SCALES[0, :2] = [1.0, 1e-06]      # nmos  m, w
SCALES[1, :2] = [1.0, 1e-06]      # pmos  m, w
SCALES[2, 0] = 1.0                # balun rout
SCALES[3, 0] = 1000.0             # resistor r
SCALES[4, 0] = 1e-12              # capacitor c
SCALES[5, 0] = 1e-09              # inductor l
SCALES[6, :3] = [1.0, 1.0, 1.0]   # vsource dc, mag, phase
SCALES[7, :2] = [0.001, 0.001]    # isource dc, mag
SCALES[8, :4] = [1.0, 1.0, 1e9, 1.0]  # port dbm, dc, freq, num

KX = 5                            # x rows: xT(4) + ones
_F32 = mybir.dt.float32
_F32R = mybir.dt.float32r
_BF16 = mybir.dt.bfloat16
_WARM_BURST = int(os.environ.get("EDGEENC_WARM_BURST", "20"))

_PROGRAM_CACHE: dict = {}
LAST_RESULT = None  # BassKernelResults of the most recent run (for test harness)


def _layout(type_ids, source_ids):
    """Sort edges by (expert, type, source); deal each (t,tid,sid) group
    round-robin over cores.  Returns:
      ORD    [NCORES, L] global edge index per slot (-1 = pad)
      m_t    [T] per-core expert segment sizes (multiples of GRP)
      ranges list of (c0, c1, pair_col) bias runs, identical on all cores
             (pair_col 0 = pad columns, skipped by the device entirely)
      pairs  list of (t, tid, sid) in pair_col order (col 0 = zero bias)
      L      padded per-core column count (multiple of BLOCK)
    """
    base_ids = BASE_MAP[type_ids]
    m_t = np.zeros(T, dtype=np.int64)
    cols = []          # per expert: list of (k_p, pair_id, idx array)
    pairs = [None]     # pair_col 0 reserved for zero bias (pad columns)
    ranges = []
    ORD_parts = []
    off = 0
    for t in range(T):
        sel = np.nonzero(base_ids == t)[0]
        if sel.shape[0] == 0:
            continue
        key = type_ids[sel].astype(np.int64) * N_SRC + source_ids[sel]
        order = np.argsort(key, kind="stable")
        sel = sel[order]
        key = key[order]
        uk, counts = np.unique(key, return_counts=True)
        seg = 0
        pos = 0
        ord_t = []
        for u, n in zip(uk, counts):
            k_p = math.ceil(n / NCORES)
            pcol = len(pairs)
            pairs.append((t, int(u) // N_SRC, int(u) % N_SRC))
            idx = np.full(NCORES * k_p, -1, dtype=np.int64)
            idx[:n] = sel[pos : pos + n]
            pos += n
            # slot j of core c <- idx[j*NCORES + c]
            ord_t.append(idx.reshape(k_p, NCORES).T)     # [NCORES, k_p]
            ranges.append((off + seg, off + seg + k_p, pcol))
            seg += k_p
        m = math.ceil(seg / GRP) * GRP
        if m > seg:
            ranges.append((off + seg, off + m, 0))
            ord_t.append(np.full((NCORES, m - seg), -1, dtype=np.int64))
        m_t[t] = m
        ORD_parts.append(np.concatenate(ord_t, axis=1))
        off += m
    L0 = off
    L = math.ceil(L0 / BLOCK) * BLOCK
    if L > L0:
        ranges.append((L0, L, 0))
        ORD_parts.append(np.full((NCORES, L - L0), -1, dtype=np.int64))
        m_t[np.nonzero(m_t)[0][-1]] += L - L0
    ORD = np.concatenate(ORD_parts, axis=1)
    return ORD, m_t, ranges, pairs, L


def _host_inputs(type_ids, source_ids, params, ORD):
    """INX[c] = [5, L] bf16: xT (scaled/masked) + ones row."""
    base_ids = BASE_MAP[type_ids]
    scales = SCALES[base_ids]                                  # [E,4]
    validp = np.arange(P_MAX)[None, :] < PCOUNT[base_ids][:, None]
    x = np.where(validp, params.astype(np.float32) / scales, 0.0).astype(np.float32)

    L = ORD.shape[1]
    INX = np.zeros((NCORES, KX, L), dtype=np.float32)
    valid = ORD >= 0
    ids = ORD[valid]
    tmp = np.zeros((NCORES, L, P_MAX), dtype=np.float32)
    tmp[valid] = x[ids]
    INX[:, 0:P_MAX, :] = tmp.transpose(0, 2, 1)
    INX[:, P_MAX, :] = valid
    return INX.astype(ml_dtypes.bfloat16)


def _host_weights(type_embed, source_embed, W1, b1, W2, b2, Wf, bf, pairs):
    f = np.float32
    W1 = W1.astype(f); b1 = b1.astype(f); W2 = W2.astype(np.float64)
    b2 = b2.astype(np.float64); Wf = Wf.astype(np.float64); bf = bf.astype(np.float64)
    type_embed = type_embed.astype(np.float64)
    source_embed = source_embed.astype(np.float64)

    # layer1 lhsT blocks: [5, 9*256]; block t at cols [t*256,(t+1)*256)
    W1e = np.concatenate([W1, b1.astype(f)[:, None, :]], axis=1)   # [9,5,256]
    W1E = np.ascontiguousarray(
        W1e.transpose(1, 0, 2).reshape(KX, T * D)).astype(ml_dtypes.bfloat16)

    Wft, Wfs, Wfp = Wf[:D], Wf[D : 2 * D], Wf[2 * D :]

    # V[t] = W2[t] @ Wf_pv (f64), fusing layer 2 with the final projection.
    # bf16 lhsT blocks: [128, 18*256]; block (t,h) = V[t][h*128:(h+1)*128,:]
    V = (W2 @ Wfp).astype(f)                                        # [9,256,256]
    VR = np.ascontiguousarray(
        V.reshape(T, 2, 128, D).transpose(2, 0, 1, 3).reshape(128, T * 2 * D)
    ).astype(ml_dtypes.bfloat16)

    # bias table: C[pair=(t,tid,sid)] = te[tid]@Wft + se[sid]@Wfs + b2[t]@Wfp
    # + bf, stored transposed [128, 2*NPC] (half g at cols g*NPC + pcol)
    NPC = len(pairs)
    CG = np.zeros((128, 2 * NPC), dtype=f)
    for pcol in range(1, NPC):
        t, tid, sid = pairs[pcol]
        c = (type_embed[tid] @ Wft + source_embed[sid] @ Wfs
             + b2[t] @ Wfp + bf).astype(f)                          # [256]
        CG[:, pcol] = c[0:128]
        CG[:, NPC + pcol] = c[128:256]
    return W1E, VR, CG


def _build_program(m_t: tuple, L: int, ranges: tuple, NPC: int):
    """One compiled SPMD program for the given segment/bias-run layout."""
    key = (m_t, L, ranges, NPC, _WARM_BURST)
    if key in _PROGRAM_CACHE:
        return _PROGRAM_CACHE[key]

    group_expert = np.repeat(np.arange(T), (np.asarray(m_t) // GRP))
    NB = L // BLOCK
    GP = BLOCK // GRP  # groups per block = 4

    nc = bacc.Bacc("TRN2", target_bir_lowering=False, debug=False,
                   num_devices=NCORES)
    inx_d = nc.dram_tensor("inx", [KX, L], _BF16, kind="ExternalInput")
    w1e_d = nc.dram_tensor("w1e", [KX, T * D], _BF16, kind="ExternalInput")
    vr_d = nc.dram_tensor("vr", [128, T * 2 * D], _BF16, kind="ExternalInput")
    cg_d = nc.dram_tensor("cg", [128, 2 * NPC], _F32, kind="ExternalInput")
    out_d = nc.dram_tensor("out", [D, L], _BF16, kind="ExternalOutput")

    RELU = mybir.ActivationFunctionType.Relu
    ADD = mybir.AluOpType.add
    MAX = mybir.AluOpType.max

    with tile.TileContext(nc) as tc:
        with (
            tc.tile_pool(name="wts", bufs=1) as wts,
            tc.tile_pool(name="inp", bufs=1) as inp,
            tc.tile_pool(name="hsb", bufs=6) as hsbp,
            tc.tile_pool(name="osb", bufs=6) as osbp,
            tc.tile_pool(name="hps", bufs=4, space=bass.MemorySpace.PSUM) as hps,
            tc.tile_pool(name="ops", bufs=4, space=bass.MemorySpace.PSUM) as ops,
        ):
            # HAM warm-up burst first: two tiny memsets at the head of the
            # vector queue, then bf16 matmuls into a scratch PSUM bank that
            # un-throttle the PE clock while the input DMAs are in flight
            if _WARM_BURST:
                wmw = wts.tile([128, 128], _BF16)
                wma = wts.tile([128, 256], _BF16)
                nc.vector.memset(wmw[:], 0.0)
                nc.vector.memset(wma[:], 0.0)
                wmp = hps.tile([PDIM, BLOCK], _F32, name="warmps", tag="hts")
                for i in range(_WARM_BURST):
                    nc.tensor.matmul(wmp[:, 0:256], wmw[:], wma[:], start=True,
                                     stop=True)

            w1e = wts.tile([128, T * D], _BF16)
            vr = wts.tile([128, T * 2 * D], _BF16)
            cg = wts.tile([128, 2 * NPC], _F32)
            # keep the initial-scope DMA volume tiny: everything the first
            # blocks need is ~250KB, so the first matmul isn't stuck behind
            # a megabyte of weight traffic
            nc.gpsimd.memset(w1e[:], 0.0)
            nc.gpsimd.dma_start(w1e[0:KX, :], w1e_d.ap())
            nc.gpsimd.dma_start(cg[:], cg_d.ap())

            # V (1.13MB bf16) loads lazily, one 128KB expert chunk at a
            # time, issued ~3 blocks before that expert's first block on
            # the otherwise-idle sync/scalar queues
            ext_end = np.cumsum(np.asarray(m_t))
            ext_start = ext_end - np.asarray(m_t)
            vr_issue = [[] for _ in range(NB)]
            for t in range(T):
                if m_t[t] == 0:
                    continue
                fb = max(0, int(ext_start[t]) // BLOCK - 3)
                vr_issue[fb].append(t)

            def load_vr(t, qi):
                eng = nc.sync if qi % 2 == 0 else nc.scalar
                eng.dma_start(vr[:, 2 * t * D : 2 * (t + 1) * D],
                              vr_d.ap()[:, 2 * t * D : 2 * (t + 1) * D])

            for qi, t in enumerate(vr_issue[0]):
                load_vr(t, qi)

            # persistent input buffers; zero-pad rows [KX:128) once so the
            # layer-1 matmul can run full-row (pads split over two engines)
            NIB = min(8, NB)
            PREF = min(4, NIB)
            xts = [inp.tile([128, BLOCK], _BF16, name=f"xtile{j}", tag=f"xtile{j}")
                   for j in range(NIB)]
            for b in range(min(PREF, NB)):
                nc.vector.memset(xts[b][:], 0.0)
                nc.gpsimd.dma_start(
                    xts[b][0:KX, :], inx_d.ap()[:, b * BLOCK : (b + 1) * BLOCK])
            for j in range(PREF, NIB):
                nc.vector.memset(xts[j][:], 0.0)

            # bias runs clipped per block; pad runs (pcol 0) are skipped and
            # also clipped out of the matmuls/evacuation/stores below
            block_ranges = [[] for _ in range(NB)]
            for (c0, c1, pcol) in ranges:
                if pcol == 0:
                    continue
                b0, b1 = c0 // BLOCK, (c1 - 1) // BLOCK
                for b in range(b0, b1 + 1):
                    r0 = max(c0, b * BLOCK) - b * BLOCK
                    r1 = min(c1, (b + 1) * BLOCK) - b * BLOCK
                    block_ranges[b].append((r0, r1, pcol))
            # pad groups: GRP-col groups lying fully inside a pad range
            is_pad = np.zeros(L // GRP, dtype=bool)
            for (c0, c1, pcol) in ranges:
                if pcol == 0:
                    g0p = (c0 + GRP - 1) // GRP
                    for g in range(g0p, c1 // GRP):
                        is_pad[g] = True

            evac_rr = 0  # round-robin bias-relu ops across Scalar/Vector
            for b in range(NB):
                g0 = b * GP
                experts = [int(group_expert[g0 + i]) for i in range(GP)]
                runs = []
                for i, t in enumerate(experts):
                    if runs and runs[-1][0] == t:
                        runs[-1] = (t, runs[-1][1], (i + 1) * GRP)
                    else:
                        runs.append((t, i * GRP, (i + 1) * GRP))
                # clip trailing pad groups out of each run (drop empty runs)
                cruns = []
                for (t, c0, c1) in runs:
                    while c1 > c0 and is_pad[g0 + c1 // GRP - 1]:
                        c1 -= GRP
                    if c1 > c0:
                        cruns.append((t, c0, c1))
                runs = cruns
                if not runs:
                    continue
                vend = max(c1 for (_, _, c1) in runs)

                xt = xts[b % NIB]
                bp = b + PREF
                if bp < NB:
                    xtp = xts[bp % NIB]
                    nc.gpsimd.dma_start(
                        xtp[0:KX, :],
                        inx_d.ap()[:, bp * BLOCK : (bp + 1) * BLOCK])
                if b > 0:
                    for qi, t in enumerate(vr_issue[b]):
                        load_vr(t, b + qi)

                # ---- layer 1: hT[h] = relu(W1e[t].T @ xT1), fp32r ----
                hts = [hps.tile([PDIM, BLOCK], _F32, name=f"hts{b}_{j}", tag="hts")
                       for j in range(2)]
                for (t, c0, c1) in runs:
                    for h in range(2):
                        nc.tensor.matmul(
                            hts[h][:, c0:c1],
                            w1e[:, t * D + h * PDIM : t * D + (h + 1) * PDIM],
                            xt[:, c0:c1],
                            start=True, stop=True,
                        )
                hsb = [hsbp.tile([PDIM, BLOCK], _BF16, name=f"hsb{b}_{j}", tag="hsb")
                       for j in range(2)]
                nc.scalar.activation(hsb[0][:, 0:vend], hts[0][:, 0:vend], RELU)
                nc.vector.tensor_scalar_max(hsb[1][:, 0:vend], hts[1][:, 0:vend],
                                            0.0)

                # ---- V stage: outT[g] += V[t]-chunk(h).T @ hT[h], bf16 ----
                ots = [ops.tile([PDIM, BLOCK], _F32, name=f"ots{b}_{j}", tag="ots")
                       for j in range(2)]
                for (t, c0, c1) in runs:
                    for h in range(2):
                        for g in range(2):
                            nc.tensor.matmul(
                                ots[g][:, c0:c1],
                                vr[:, (t * 2 + h) * D + g * PDIM
                                   : (t * 2 + h) * D + (g + 1) * PDIM],
                                hsb[h][:, c0:c1],
                                start=(h == 0), stop=(h == 1),
                            )

                # ---- bias-relu evacuation: out = relu(psum + C[pair]) ----
                osb = [osbp.tile([PDIM, BLOCK], _BF16, name=f"osb{b}_{j}", tag="osb")
                       for j in range(2)]
                for (r0, r1, pcol) in block_ranges[b]:
                    for g in range(2):
                        bias_ap = cg[:, g * NPC + pcol : g * NPC + pcol + 1]
                        if evac_rr % 2 == 0:
                            nc.scalar.activation(
                                osb[g][:, r0:r1], ots[g][:, r0:r1], RELU,
                                bias=bias_ap)
                        else:
                            nc.vector.tensor_scalar(
                                osb[g][:, r0:r1], ots[g][:, r0:r1],
                                bias_ap, 0.0, op0=ADD, op1=MAX)
                        evac_rr += 1
                for g in range(2):
                    nc.sync.dma_start(
                        out_d.ap()[g * PDIM : (g + 1) * PDIM,
                                   b * BLOCK : b * BLOCK + vend],
                        osb[g][:, 0:vend],
                    )

    nc.compile()
    _PROGRAM_CACHE[key] = nc
    return nc


def kernel(type_ids, source_ids, params, type_embed, source_embed,
           W1, b1, W2, b2, Wf, bf):
    global LAST_RESULT
    type_ids = np.asarray(type_ids, dtype=np.int32)
    source_ids = np.asarray(source_ids, dtype=np.int32)
    params = np.asarray(params, dtype=np.float32)
    E = type_ids.shape[0]

    ORD, m_t, ranges, pairs, L = _layout(type_ids, source_ids)
    INX = _host_inputs(type_ids, source_ids, params, ORD)
    W1E, VR, CG = _host_weights(
        np.asarray(type_embed), np.asarray(source_embed),
        np.asarray(W1), np.asarray(b1), np.asarray(W2), np.asarray(b2),
        np.asarray(Wf), np.asarray(bf), pairs)

    nc = _build_program(tuple(int(v) for v in m_t), L,
                        tuple(ranges), len(pairs))

    in_maps = [{"inx": np.ascontiguousarray(INX[c]), "w1e": W1E, "vr": VR,
                "cg": CG} for c in range(NCORES)]

    trace = bool(int(os.environ.get("EDGEENC_TRACE", "0")))
    res = run_bass_kernel_spmd(nc, in_maps, core_ids=list(range(NCORES)),
                               trace=trace)
    LAST_RESULT = res

    full = np.zeros((E, D), dtype=np.float32)
    for c in range(NCORES):
        sel = ORD[c] >= 0
        oc = res.results[c]["out"]                     # [D, L] bf16
        full[ORD[c][sel]] = np.ascontiguousarray(
            oc[:, sel].T).astype(np.float32)
    return full
